# revision 1
# baseline (speedup 1.0000x reference)
"""Multi-head attention Trainium2 kernel, 8-core SPMD.

Problem: x[2,4096,512], 8 heads of 64; per-head QKV proj, softmax(QK^T/8)V,
concat, output proj.

Sharding: sequence-parallel, no collectives. Core c handles batch b=c//4 and
query rows [1024*(c%4), 1024*(c%4)+1024). Each core computes K/V for the full
4096-row sequence of its batch (4x duplicated work, hidden under the ACT exp
bottleneck) and writes its own 1024x512 output slice.

Layouts (SBUF, partition dim first):
  xT   [128,4,512]   x^T chunk: partition=d%128, dsub=d//128, free=t_local
  kT   [128,4,4096]  bf16 K^T: partition p,group g -> row g*128+p = h*64+e
  qT   [128,4,1024]  bf16 Q^T, same row packing, local q cols
  v    [128,32,8,65] bf16 V augmented: [t%128, t//128, h, e(+ones col 64)]
  yT   [128,4,1024]  fp32 attention out^T, rows (h,e), local q cols
Scores are computed transposed (S^T[t,s]) so softmax needs no transposes:
exp on ACT reads score PSUM directly; the ones-column of V makes row 64 of
the PV accumulation equal the softmax denominator.
"""

import numpy as np

import concourse.bass as bass
from concourse import bacc
import concourse.mybir as mybir
import concourse.tile as tile
from concourse.bass_utils import run_bass_kernel_spmd

F32 = mybir.dt.float32
F32R = mybir.dt.float32r
BF16 = mybir.dt.bfloat16

B, S, D, H, E = 2, 4096, 512, 8, 64
NCORES = 8
QCHUNK = S // 4          # 1024 query rows per core
TCH = 512                # t-rows per phase-1 chunk
G = 3                    # score psum banks per exp instruction


def build_program():
    nc = bacc.Bacc()
    xt_d = nc.dram_tensor("xt", [D, S], F32R, kind="ExternalInput")
    wq_d = nc.dram_tensor("wq", [128, 4, 512], F32R, kind="ExternalInput")
    wk_d = nc.dram_tensor("wk", [128, 4, 512], F32R, kind="ExternalInput")
    wv_d = nc.dram_tensor("wv", [128, 4, 512], F32R, kind="ExternalInput")
    wo_d = nc.dram_tensor("wo", [128, 4, 512], F32R, kind="ExternalInput")
    bq_d = nc.dram_tensor("bq", [128, 4], F32, kind="ExternalInput")
    bk_d = nc.dram_tensor("bk", [128, 4], F32, kind="ExternalInput")
    bv_d = nc.dram_tensor("bv", [512], F32, kind="ExternalInput")
    bo_d = nc.dram_tensor("bo", [512], F32, kind="ExternalInput")
    out_d = nc.dram_tensor("out", [QCHUNK, D], F32, kind="ExternalOutput")

    # q0 is passed per-core but we cannot branch on it cheaply; instead each
    # core gets its own x already rolled so its query rows sit at rows 0:1024.
    # (host side rolls x; kernel always uses rows 0:1024 as queries)

    with tile.TileContext(nc) as tc:
        with (
            tc.tile_pool(name="const", bufs=1) as cpool,
            tc.tile_pool(name="work", bufs=3) as wpool,
            tc.tile_pool(name="pt", bufs=8) as ptpool,
            tc.tile_pool(name="ps", bufs=2, space="PSUM") as pspool,
            tc.tile_pool(name="dr", bufs=2, space="DRAM") as dpool,
        ):
            wq_s = cpool.tile([128, 4, 512], F32R, tag="wq")
            wk_s = cpool.tile([128, 4, 512], F32R, tag="wk")
            wv_s = cpool.tile([128, 4, 512], F32R, tag="wv")
            wo_s = cpool.tile([128, 4, 512], F32R, tag="wo")
            bq_s = cpool.tile([128, 4], F32, tag="bq")
            bk_s = cpool.tile([128, 4], F32, tag="bk")
            bv_r = cpool.tile([128, 512], F32, tag="bvr")
            bo_r = cpool.tile([128, 512], F32, tag="bor")
            nc.sync.dma_start(wq_s[:], wq_d[:])
            nc.sync.dma_start(wk_s[:], wk_d[:])
            nc.sync.dma_start(wv_s[:], wv_d[:])
            nc.sync.dma_start(wo_s[:], wo_d[:])
            nc.sync.dma_start(bq_s[:], bq_d[:])
            nc.sync.dma_start(bk_s[:], bk_d[:])
            nc.sync.dma_start(bv_r[:], bv_d[:].unsqueeze(0).to_broadcast((128, 512)))
            nc.sync.dma_start(bo_r[:], bo_d[:].unsqueeze(0).to_broadcast((128, 512)))

            kT = cpool.tile([128, 4, S], BF16, tag="kT")
            qT = cpool.tile([128, 4, QCHUNK], BF16, tag="qT")
            vA = cpool.tile([128, S // 128, H, E + 1], BF16, tag="vA")
            yT = cpool.tile([128, 4, QCHUNK], F32R, tag="yT")
            nc.vector.memset(vA[:, :, :, E], 1.0)

            # ---- phase 1: x -> xT chunks -> K^T, V, Q^T projections ----
            for ch in range(S // TCH):
                xT = wpool.tile([128, 4, TCH], F32R, tag="xT")
                for ds_ in range(4):
                    nc.sync.dma_start(
                        xT[:, ds_, :],
                        xt_d[ds_ * 128 : (ds_ + 1) * 128, ch * TCH : (ch + 1) * TCH],
                    )
                # K^T rows: 4 groups of 128
                for eg in range(4):
                    pk = pspool.tile([128, 512], F32, tag="small")
                    for ds_ in range(4):
                        nc.tensor.matmul(
                            pk[:, :TCH],
                            wk_s[:, ds_, eg * 128 : (eg + 1) * 128],
                            xT[:, ds_, :],
                            start=(ds_ == 0),
                            stop=(ds_ == 3),
                        )
                    nc.vector.tensor_tensor(
                        out=kT[:, eg, ch * TCH : (ch + 1) * TCH],
                        in0=pk[:, :TCH],
                        in1=bk_s[:, eg, None].to_broadcast([128, TCH]),
                        op=mybir.AluOpType.add,
                    )
                # V rows (t on partitions)
                for ts_ in range(TCH // 128):
                    pv = pspool.tile([128, 512], F32, tag="small")
                    for ds_ in range(4):
                        nc.tensor.matmul(
                            pv[:],
                            xT[:, ds_, ts_ * 128 : (ts_ + 1) * 128],
                            wv_s[:, ds_, :],
                            start=(ds_ == 0),
                            stop=(ds_ == 3),
                        )
                    nc.vector.tensor_tensor(
                        out=vA[:, ch * 4 + ts_, :, 0:E],
                        in0=pv[:].rearrange("p (h e) -> p h e", h=H),
                        in1=bv_r[:].rearrange("p (h e) -> p h e", h=H),
                        op=mybir.AluOpType.add,
                    )
                # Q^T for query chunks (local rows 0:1024 of this core's x)
                if ch < QCHUNK // TCH:
                    for eg in range(4):
                        pq = pspool.tile([128, 512], F32, tag="small")
                        for ds_ in range(4):
                            nc.tensor.matmul(
                                pq[:, :TCH],
                                wq_s[:, ds_, eg * 128 : (eg + 1) * 128],
                                xT[:, ds_, :],
                                start=(ds_ == 0),
                                stop=(ds_ == 3),
                            )
                        nc.vector.tensor_tensor(
                            out=qT[:, eg, ch * TCH : (ch + 1) * TCH],
                            in0=pq[:, :TCH],
                            in1=bq_s[:, eg, None].to_broadcast([128, TCH]),
                            op=mybir.AluOpType.add,
                        )

            # ---- phase 2: attention per head / 512-wide query chunk ----
            NT = S // 128          # 32 t-tiles
            for hp in range(H // 2):
                g = hp
                for sc in range(QCHUNK // 512):
                    pav0 = pspool.tile([128, 512], F32, tag="av")
                    pav1 = pspool.tile([128, 512], F32, tag="av")
                    for tt in range(NT):
                        psc = pspool.tile([128, 2, 512], F32, tag="sc")
                        for hh in range(2):
                            p0 = hh * 64
                            nc.tensor.matmul(
                                psc[:, hh, :],
                                kT[p0 : p0 + 64, g, tt * 128 : (tt + 1) * 128],
                                qT[p0 : p0 + 64, g, sc * 512 : (sc + 1) * 512],
                                start=True,
                                stop=True,
                            )
                        pt = ptpool.tile([128, 2, 512], BF16, tag="pt")
                        nc.scalar.activation(
                            pt[:],
                            psc[:],
                            mybir.ActivationFunctionType.Exp,
                            scale=0.125,
                        )
                        for hh, pav in ((0, pav0), (1, pav1)):
                            nc.tensor.matmul(
                                pav[0:65, :],
                                vA[:, tt, 2 * hp + hh, :],
                                pt[:, hh, :],
                                start=(tt == 0),
                                stop=(tt == NT - 1),
                            )
                    for hh, pav in ((0, pav0), (1, pav1)):
                        p0 = hh * 64
                        avs = wpool.tile([65, 512], F32, tag="avs")
                        nc.vector.tensor_copy(avs[:], pav[0:65, :])
                        rec = wpool.tile([1, 512], F32, tag="rec")
                        nc.vector.reciprocal(rec[:], avs[64:65, :])
                        rrep = wpool.tile([64, 512], F32, tag="rrep")
                        rec_d = dpool.tile([1, 512], F32, tag="recd")
                        nc.sync.dma_start(rec_d[:], rec[:])
                        nc.sync.dma_start(rrep[:], rec_d[:].to_broadcast((64, 512)))
                        nc.vector.tensor_tensor(
                            out=yT[p0 : p0 + 64, g, sc * 512 : (sc + 1) * 512],
                            in0=avs[0:64, :],
                            in1=rrep[:],
                            op=mybir.AluOpType.mult,
                        )

            # ---- phase 3: output projection ----
            for st in range(QCHUNK // 128):
                po = pspool.tile([128, 512], F32, tag="small")
                for g in range(4):
                    nc.tensor.matmul(
                        po[:],
                        yT[:, g, st * 128 : (st + 1) * 128],
                        wo_s[:, g, :],
                        start=(g == 0),
                        stop=(g == 3),
                    )
                o_s = wpool.tile([128, 512], F32, tag="osb")
                nc.vector.tensor_tensor(o_s[:], po[:], bo_r[:], mybir.AluOpType.add
                )
                nc.sync.dma_start(
                    out_d[st * 128 : (st + 1) * 128, :], o_s[:]
                )
    nc.compile()
    return nc


_NC = None


def kernel(x, Wq, bq, Wk, bk, Wv, bv, Wo, bo, **kw):
    global _NC
    x = np.asarray(x, np.float32)
    s = lambda a: np.ascontiguousarray(np.asarray(a, np.float32))
    # weight packing shared by all cores
    wq_p = s(np.transpose(Wq, (1, 0, 2)).reshape(D, 512).reshape(4, 128, 512)
             .transpose(1, 0, 2))
    wk_p = s(np.transpose(Wk, (1, 0, 2)).reshape(D, 512).reshape(4, 128, 512)
             .transpose(1, 0, 2))
    wv_p = s(np.transpose(Wv, (1, 0, 2)).reshape(D, 512).reshape(4, 128, 512)
             .transpose(1, 0, 2))
    wo_p = s(np.asarray(Wo, np.float32).reshape(4, 128, 512).transpose(1, 0, 2))
    bq_p = s(np.asarray(bq, np.float32).reshape(512).reshape(4, 128).T)
    bk_p = s(np.asarray(bk, np.float32).reshape(512).reshape(4, 128).T)
    bv_p = s(np.asarray(bv, np.float32).reshape(512))
    bo_p = s(np.asarray(bo, np.float32))

    if _NC is None:
        _NC = build_program()

    in_maps = []
    for c in range(NCORES):
        b = c // 4
        q0 = (c % 4) * QCHUNK
        xb = np.roll(x[b], -q0, axis=0)  # queries at rows 0:1024
        in_maps.append({
            "xt": np.ascontiguousarray(xb.T),
            "wq": wq_p, "wk": wk_p, "wv": wv_p, "wo": wo_p,
            "bq": bq_p, "bk": bk_p, "bv": bv_p, "bo": bo_p,
        })
    res = run_bass_kernel_spmd(_NC, in_maps, core_ids=list(range(NCORES)))
    out = np.empty((B, S, D), np.float32)
    for c in range(NCORES):
        b = c // 4
        q0 = (c % 4) * QCHUNK
        out[b, q0 : q0 + QCHUNK] = res.results[c]["out"]
    return out



# revision 26
# speedup vs baseline: 1.6978x; 1.6978x over previous
"""Multi-head attention Trainium2 kernel, 8-core SPMD, head+batch sharded.

Problem: x[2,4096,512], 8 heads of 64; per-head QKV proj, softmax(QK^T/8)V,
concat, output proj.

Sharding: core c = b*4 + hp handles batch b and head-pair (2hp, 2hp+1) over
the full 4096x4096 attention. Each core emits a PARTIAL output projection
(rows of Wo for its two heads); the host gather sums the 4 partials per batch
and adds bo (the all-reduce of the row-sharded projection done at unshard).

Structure (v3, sequencer/PSUM-aware):
  phase 1: x resident in SBUF (32KB bf16); K/Q/V projected per 512-t chunk
    (bf16 matmuls, 4-step d-contraction). K/Q evacuated by ACT (fused
    per-partition bias + fp8e4m3 convert), V by DVE (free-dim bias via
    tensor_tensor). K/Q then DMA-rearranged (Pool SWDGE, per-2048 blocks)
    into the DoubleRow layout [e%32][eh][h][t].
  phase 2: two head-phased sweeps (h=0 then h=1) over q-chunks of 512,
    t-pairs of 256 inner: scores = one fp8 DoubleRow matmul per 128-t tile
    (contraction 2x32); exp split ACT (exact, ->fp8) / DVE (Schraudolph:
    int8(z*8/ln2 + 55.65) bitcast fp8e4m3) -- Pool cannot read PSUM; AV via
    fp8 DoubleRow (contraction 2x128), V's ones-column accumulating the
    softmax denominator at PSUM row 64. Tail per (h,qc): DVE reciprocal +
    broadcast-multiply, PE transpose (identity matmul, h1 at tile_position
    col 64), DVE evac to yT8[he,q]; in the h1 sweep also the partial
    out-projection (contraction 128) + ACT evac + SP DMA out.

PSUM (8 banks): tag "sc" 3x[128,2,512]f32 (6 banks; also rotates phase-1
proj PSUM and tail ytp/po tiles) + tag "pav" 2x[128,4,65]f32 (2 banks).
"""

import numpy as np
import ml_dtypes

import concourse.bass as bass
from concourse import bacc
import concourse.mybir as mybir
import concourse.tile as tile
from concourse.bass_utils import run_bass_kernel_spmd

F32 = mybir.dt.float32
BF16 = mybir.dt.bfloat16
F8E4 = mybir.dt.float8e4
I8 = mybir.dt.int8
NBF = ml_dtypes.bfloat16

B, S, D, H, E = 2, 4096, 512, 8, 64
NCORES = 8
TCH = 512
NCH = S // TCH            # 8 chunks
NTP = S // 256            # 16 t-pairs per sweep
NQC = S // 512            # 8 q-chunks
LN2 = float(np.log(2.0))
SCH_MULT = 0.125 * 8.0 / LN2
SCH_BIAS = (7.0 - 0.0435) * 8.0

DR = mybir.MatmulPerfMode.DoubleRow
Exp = mybir.ActivationFunctionType.Exp
Ident = mybir.ActivationFunctionType.Identity
ADD = mybir.AluOpType.add
MULT = mybir.AluOpType.mult

# exp instructions per sweep assigned ACT vs DVE (weighted by per-instr cost
# 1038ns vs 1192ns, minus fixed non-exp engine load in each sweep)
ACT_N = {0: 74, 1: 65}
# V-projection pieces (2 of the 32 t-subtiles each) paced ~1.5 t-pairs apart
V_AT = {}
for _p in range(16):
    V_AT.setdefault(min(int(1.5 * _p), _p + 5), []).append(_p)


def _sweep_sched(n_act, total=128):
    acc_a = acc_d = 0.0
    fa, fd = n_act / total, (total - n_act) / total
    out = []
    for _ in range(total):
        acc_a += fa
        acc_d += fd
        if acc_a >= acc_d:
            out.append("A")
            acc_a -= 1.0
        else:
            out.append("D")
            acc_d -= 1.0
    return out


def build_program():
    nc = bacc.Bacc()
    xt_d = nc.dram_tensor("xt", [D, S], BF16, kind="ExternalInput")
    wq_d = nc.dram_tensor("wq", [128, 4, 128], BF16, kind="ExternalInput")
    wk_d = nc.dram_tensor("wk", [128, 4, 128], BF16, kind="ExternalInput")
    wv_d = nc.dram_tensor("wv", [128, 4, 128], BF16, kind="ExternalInput")
    wo_d = nc.dram_tensor("wo", [128, 512], BF16, kind="ExternalInput")
    bq_d = nc.dram_tensor("bq", [128, 1], F32, kind="ExternalInput")
    bk_d = nc.dram_tensor("bk", [128, 1], F32, kind="ExternalInput")
    bv_d = nc.dram_tensor("bv", [128], F32, kind="ExternalInput")
    id_d = nc.dram_tensor("idm", [128, 128], BF16, kind="ExternalInput")
    out_d = nc.dram_tensor("out", [S, D], F32, kind="ExternalOutput")

    with tile.TileContext(nc) as tc:
        with (
            tc.tile_pool(name="const", bufs=1) as cpool,
            tc.tile_pool(name="ptp", bufs=10) as ptpool,
            tc.tile_pool(name="wk2", bufs=2) as wpool,
            tc.tile_pool(name="ps", bufs=3, space="PSUM") as pspool,
        ):
            wq_s = cpool.tile([128, 4, 128], BF16, tag="wq")
            wk_s = cpool.tile([128, 4, 128], BF16, tag="wk")
            wv_s = cpool.tile([128, 4, 128], BF16, tag="wv")
            wo_s = cpool.tile([128, 512], BF16, tag="wo")
            bq_s = cpool.tile([128, 1], F32, tag="bq")
            bk_s = cpool.tile([128, 1], F32, tag="bk")
            bv_r = cpool.tile([128, 128], F32, tag="bvr")
            id_s = cpool.tile([128, 128], BF16, tag="idm")
            dum = cpool.tile([1, 4], F32, tag="dum")
            nc.vector.memset(dum[:], 0.0)
            nc.scalar.activation(dum[:], dum[:], Exp)  # hoist act-table load
            nc.sync.dma_start(wk_s[:], wk_d[:])
            nc.sync.dma_start(bk_s[:], bk_d[:])

            xA = cpool.tile([128, 4, S], BF16, tag="xA")
            kT2 = cpool.tile([128, S], F8E4, tag="kT2")
            qT2 = cpool.tile([128, S], F8E4, tag="qT2")
            kT8 = cpool.tile([32, 2, 2, S], F8E4, tag="kT8")
            qT8 = cpool.tile([32, 2, 2, S], F8E4, tag="qT8")
            vA = cpool.tile([128, NTP, 2, 2, E + 1], F8E4, tag="vA")
            yT8 = cpool.tile([128, NQC, 512], BF16, tag="yT8")
            nc.vector.memset(vA[:, :, :, :, E], 1.0)
            zro = cpool.tile([1, 260], BF16, tag="zro")
            nc.vector.memset(zro[:], 0.0)

            # ---- projections (K up front; Q/V interleaved into the sweep) --
            xt_v = xt_d[:].rearrange("(a b) t -> b a t", a=4)  # [128,4,S] view

            def kq_proj(ch, w_s, b_s, dst, pair=False):
                t0 = ch * TCH
                pk = pspool.tile([128, 2, 512], F32, tag="sc",
                                 name=f"pkq{ch}")
                n = 2 if pair else 1
                for i in range(n):
                    for ds_ in range(4):
                        nc.tensor.matmul(
                            pk[:, i, :], w_s[:, ds_, :],
                            xA[:, ds_, t0 + i * TCH:t0 + (i + 1) * TCH],
                            start=(ds_ == 0), stop=(ds_ == 3))
                nc.scalar.activation(dst[:, t0:t0 + n * TCH],
                                     pk[:, 0:n, :].rearrange("p a b -> p (a b)"),
                                     Ident, bias=b_s[:], scale=1.0)

            def rearr(src, dst, t0, tlen, eng=None):
                eng = eng or nc.sync
                for h in range(2):
                    for eh in range(2):
                        r0 = h * 64 + eh * 32
                        eng.dma_start(dst[0:32, eh, h, t0:t0 + tlen],
                                      src[r0:r0 + 32, t0:t0 + tlen])

            def v_piece(p):
                for ts_ in (2 * p, 2 * p + 1):
                    pv = pspool.tile([128, 512], F32, tag="ytp", bufs=1,
                                     name=f"pv{ts_}")
                    for ds_ in range(4):
                        nc.tensor.matmul(
                            pv[:, 0:128],
                            xA[:, ds_, ts_ * 128:(ts_ + 1) * 128],
                            wv_s[:, ds_, :], start=(ds_ == 0), stop=(ds_ == 3))
                    nc.vector.tensor_tensor(
                        out=vA[:, ts_ // 2, ts_ % 2, :, 0:E],
                        in0=pv[:, 0:128].rearrange("p (h e) -> p h e", h=2),
                        in1=bv_r[:].rearrange("p (h e) -> p h e", h=2),
                        op=ADD)

            def q_proj(ch):
                kq_proj(ch, wq_s, bq_s, qT2)
                rearr(qT2, qT8, ch * TCH, TCH)

            for ch in range(2):
                nc.sync.dma_start(xA[:, :, ch * TCH:(ch + 1) * TCH],
                                  xt_v[:, :, ch * TCH:(ch + 1) * TCH])
            nc.sync.dma_start(wq_s[:], wq_d[:])
            nc.sync.dma_start(bq_s[:], bq_d[:])
            nc.sync.dma_start(wv_s[:], wv_d[:])
            nc.sync.dma_start(
                bv_r[:], bv_d[:].unsqueeze(0).to_broadcast((128, 128)))
            nc.sync.dma_start(wo_s[:], wo_d[:])
            nc.sync.dma_start(id_s[:], id_d[:])
            for ch in range(2, NCH):
                nc.sync.dma_start(xA[:, :, ch * TCH:(ch + 1) * TCH],
                                  xt_v[:, :, ch * TCH:(ch + 1) * TCH])
            for ch in range(0, NCH, 2):
                kq_proj(ch, wk_s, bk_s, kT2, pair=True)
                rearr(kT2, kT8, ch * TCH, 2 * TCH,
                      eng=(nc.sync if ch < 4 else nc.gpsimd))
            q_proj(0)

            # ---- phase 2: two head-phased sweeps, tails lagged one qc and
            # emitted in small pieces so no engine queue parks on the chain --
            def norm_piece(h, qc, pav_box):
                pav = pav_box["t"]
                rec = wpool.tile([128, 4], F32, tag="rec", name=f"rec{h}_{qc}")
                nc.vector.reciprocal(rec[:], pav[:, :, E])
                yN = wpool.tile([128, 4, E], BF16, tag="yN", name=f"yN{h}_{qc}")
                nc.vector.tensor_tensor(
                    out=yN[:], in0=pav[:, :, 0:E],
                    in1=rec[:, :, None].to_broadcast((128, 4, E)), op=MULT)
                return yN

            def transpose_piece(h, qc, yN, qbs):
                ytp = pspool.tile([128, 2, 128], BF16, tag="ytp",
                                  bufs=1, name=f"ytp{h}_{qc}_{qbs[0]}")
                for i, qb in enumerate(qbs):
                    nc.tensor.transpose(ytp[h * 64:(h + 1) * 64, i, :],
                                        yN[:, qb, :], id_s[:],
                                        tile_position=(0, h * 64))
                nc.vector.tensor_copy(
                    yT8[h * 64:(h + 1) * 64, qc,
                        qbs[0] * 128:(qbs[0] + 2) * 128],
                    ytp[h * 64:(h + 1) * 64, :, :].rearrange("p a b -> p (a b)"))

            out_v = out_d[:].rearrange("(qb p) d -> p qb d", p=128)

            def out_piece(qc, qbs, o_s):
                po = pspool.tile([128, 2, 512], F32, tag="sc",
                                 name=f"po{qc}_{qbs[0]}")
                for i, qb in enumerate(qbs):
                    nc.tensor.matmul(po[:, i, :],
                                     yT8[:, qc, qb * 128:(qb + 1) * 128],
                                     wo_s[:], start=True, stop=True)
                nc.scalar.copy(
                    o_s[:, qbs[0]:qbs[0] + 2, :].rearrange("p a b -> p (a b)"),
                    po[:].rearrange("p a b -> p (a b)"))
                if qbs[-1] == 3:
                    nc.sync.dma_start(
                        out_v[:, qc * 4:(qc + 1) * 4, :], o_s[:])

            def tail_pieces(h, qc, pav_box):
                # generator-style: each closure runs later, in pop order
                state = {}

                def p_norm():
                    state["yN"] = norm_piece(h, qc, pav_box)

                yield p_norm
                yield lambda: transpose_piece(h, qc, state["yN"], (0, 1))
                yield lambda: transpose_piece(h, qc, state["yN"], (2, 3))
                if h == 1:
                    def p_out1():
                        state["o_s"] = wpool.tile([128, 4, 512], F32,
                                                  tag="osb", name=f"os{qc}")
                        out_piece(qc, (0, 1), state["o_s"])

                    yield p_out1
                    yield lambda: out_piece(qc, (2, 3), state["o_s"])

            pending = []  # deferred tail pieces, popped one per even tp
            avq = []      # (pav, h, pt, tp) awaiting AV emission (lag 4)

            def emit_av(pav_box, h, ptp_, tpp):
                if "t" not in pav_box:
                    pav_box["t"] = pspool.tile(
                        [128, 4, E + 1], F32, tag="pav", bufs=1,
                        name=f"pav{h}_{pav_box['qc']}")
                    # explicit zero-fill: one whole-tile start=True matmul so
                    # the per-qb accumulation groups never re-zero each other
                    nc.tensor.matmul(
                        pav_box["t"][:].rearrange("p a b -> p (a b)"),
                        zro[0:1, 0:128], zro[0:1, 0:260],
                        start=True, stop=True)
                pav = pav_box["t"]
                for qb in range(4):
                    nc.tensor.matmul(
                        pav[:, qb, :], ptp_[:, :, qb * 128:(qb + 1) * 128],
                        vA[:, tpp, :, h, :],
                        start=False, stop=(tpp == NTP - 1), perf_mode=DR)

            for h in range(2):
                sched = _sweep_sched(ACT_N[h])
                ei = 0
                for qc in range(NQC):
                    q0 = qc * 512
                    pav_box = {"qc": qc}
                    for tp in range(NTP):
                        psc = pspool.tile([128, 2, 512], F32, tag="sc",
                                          name=f"psc{h}_{qc}_{tp}")
                        for i in range(2):
                            tt = tp * 2 + i
                            nc.tensor.matmul(
                                psc[:, i, :],
                                kT8[0:32, :, h, tt * 128:(tt + 1) * 128],
                                qT8[0:32, :, h, q0:q0 + 512],
                                start=True, stop=True, perf_mode=DR)
                        pt = ptpool.tile([128, 2, 512], F8E4, tag="pt",
                                         name=f"pt{h}_{qc}_{tp}")
                        if sched[ei] == "A":
                            nc.scalar.activation(pt[:], psc[:], Exp, scale=0.125)
                        else:
                            nc.vector.tensor_scalar(
                                pt[:].bitcast(I8), psc[:], SCH_MULT, SCH_BIAS,
                                MULT, ADD)
                        ei += 1
                        gtp = qc * NTP + tp
                        if h == 0 and gtp in V_AT:
                            for p in V_AT[gtp]:
                                v_piece(p)        # V feeds AV pops (lag 8)
                        if h == 0 and qc < NQC - 1 and tp == 9:
                            q_proj(qc + 1)        # Q chunk for the next qc
                        if tp in (6, 8, 10, 12, 14) and pending:
                            pending.pop(0)()
                        if len(avq) >= 6:
                            emit_av(*avq.pop(0))
                        avq.append((pav_box, h, pt, tp))
                    pending.extend(tail_pieces(h, qc, pav_box))
            while avq:
                emit_av(*avq.pop(0))
            for fn in pending:
                fn()
    nc.compile()
    return nc


_NC = None


def kernel(x, Wq, bq, Wk, bk, Wv, bv, Wo, bo, **kw):
    global _NC
    x = np.asarray(x, np.float32)
    if _NC is None:
        _NC = build_program()

    idm = np.eye(128, dtype=NBF)
    xts = [np.ascontiguousarray(np.asarray(x[b], np.float32).T.astype(NBF))
           for b in range(B)]

    def wpack(W, hp):
        w = np.concatenate([np.asarray(W[2 * hp], np.float32),
                            np.asarray(W[2 * hp + 1], np.float32)], axis=1)
        return np.ascontiguousarray(
            w.reshape(4, 128, 128).transpose(1, 0, 2).astype(NBF))

    def bpack(bvec, hp):
        return np.ascontiguousarray(np.concatenate(
            [np.asarray(bvec[2 * hp], np.float32),
             np.asarray(bvec[2 * hp + 1], np.float32)]))

    in_maps = []
    for c in range(NCORES):
        b, hp = c // 4, c % 4
        in_maps.append({
            "xt": xts[b],
            "wq": wpack(Wq, hp), "wk": wpack(Wk, hp), "wv": wpack(Wv, hp),
            "wo": np.ascontiguousarray(
                np.asarray(Wo, np.float32)[2 * hp * 64:(2 * hp + 2) * 64]
                .astype(NBF)),
            "bq": bpack(bq, hp)[:, None], "bk": bpack(bk, hp)[:, None],
            "bv": bpack(bv, hp),
            "idm": idm,
        })
    res = run_bass_kernel_spmd(_NC, in_maps, core_ids=list(range(NCORES)))
    out = np.zeros((B, S, D), np.float32)
    for c in range(NCORES):
        out[c // 4] += res.results[c]["out"]
    out += np.asarray(bo, np.float32)[None, None, :]
    return out


# revision 38
# speedup vs baseline: 1.7593x; 1.0363x over previous
"""Multi-head attention Trainium2 kernel, 8-core SPMD, head+batch sharded.

Problem: x[2,4096,512], 8 heads of 64; per-head QKV proj, softmax(QK^T/8)V,
concat, output proj.

Sharding: core c = b*4 + hp handles batch b and head-pair (2hp, 2hp+1) over
the full 4096x4096 attention. Each core emits a PARTIAL output projection
(rows of Wo for its two heads); the host gather sums the 4 partials per batch
and adds bo (the all-reduce of the row-sharded projection done at unshard).

Structure (v3, sequencer/PSUM-aware):
  phase 1: x resident in SBUF (32KB bf16); K/Q/V projected per 512-t chunk
    (bf16 matmuls, 4-step d-contraction). K/Q evacuated by ACT (fused
    per-partition bias + fp8e4m3 convert), V by DVE (free-dim bias via
    tensor_tensor). K/Q then DMA-rearranged (Pool SWDGE, per-2048 blocks)
    into the DoubleRow layout [e%32][eh][h][t].
  phase 2: two head-phased sweeps (h=0 then h=1) over q-chunks of 512,
    t-pairs of 256 inner: scores = one fp8 DoubleRow matmul per 128-t tile
    (contraction 2x32); exp split ACT (exact, ->fp8) / DVE (Schraudolph:
    int8(z*8/ln2 + 55.65) bitcast fp8e4m3) -- Pool cannot read PSUM; AV via
    fp8 DoubleRow (contraction 2x128), V's ones-column accumulating the
    softmax denominator at PSUM row 64. Tail per (h,qc): DVE reciprocal +
    broadcast-multiply, PE transpose (identity matmul, h1 at tile_position
    col 64), DVE evac to yT8[he,q]; in the h1 sweep also the partial
    out-projection (contraction 128) + ACT evac + SP DMA out.

PSUM (8 banks): tag "sc" 3x[128,2,512]f32 (6 banks; also rotates phase-1
K/Q proj PSUM and tail po tiles) + "pav" 1 bank (zero-filled via an explicit
start=True matmul -- per-qb group starts would re-zero each other's bank) +
"ytp" 1 bank (V-proj PSUM early, transpose staging at tails).
"""

import numpy as np
import ml_dtypes

import concourse.bass as bass
from concourse import bacc
import concourse.mybir as mybir
import concourse.tile as tile
from concourse.bass_utils import run_bass_kernel_spmd

F32 = mybir.dt.float32
BF16 = mybir.dt.bfloat16
F8E4 = mybir.dt.float8e4
I8 = mybir.dt.int8
NBF = ml_dtypes.bfloat16

B, S, D, H, E = 2, 4096, 512, 8, 64
NCORES = 8
TCH = 512
NCH = S // TCH            # 8 chunks
NTP = S // 256            # 16 t-pairs per sweep
NQC = S // 512            # 8 q-chunks
LN2 = float(np.log(2.0))
SCH_MULT = 0.125 * 8.0 / LN2
SCH_BIAS = (7.0 - 0.0435) * 8.0

DR = mybir.MatmulPerfMode.DoubleRow
Exp = mybir.ActivationFunctionType.Exp
Ident = mybir.ActivationFunctionType.Identity
ADD = mybir.AluOpType.add
MULT = mybir.AluOpType.mult

# exp instructions per sweep assigned ACT vs DVE (weighted by per-instr cost
# 1038ns vs 1192ns, minus fixed non-exp engine load in each sweep)
ACT_N = {0: 74, 1: 64}
# V-projection pieces (2 of the 32 t-subtiles each) paced ~1.5 t-pairs apart
V_AT = {}
for _p in range(16):
    V_AT.setdefault(min(int(1.25 * _p), _p + 5), []).append(_p)


def _sweep_sched(n_act, total=128):
    acc_a = acc_d = 0.0
    fa, fd = n_act / total, (total - n_act) / total
    out = []
    for _ in range(total):
        acc_a += fa
        acc_d += fd
        if acc_a >= acc_d:
            out.append("A")
            acc_a -= 1.0
        else:
            out.append("D")
            acc_d -= 1.0
    return out


def build_program():
    nc = bacc.Bacc()
    xt_d = nc.dram_tensor("xt", [D, S], BF16, kind="ExternalInput")
    wq_d = nc.dram_tensor("wq", [128, 4, 128], BF16, kind="ExternalInput")
    wk_d = nc.dram_tensor("wk", [128, 4, 128], BF16, kind="ExternalInput")
    wv_d = nc.dram_tensor("wv", [128, 4, 128], BF16, kind="ExternalInput")
    wo_d = nc.dram_tensor("wo", [128, 512], BF16, kind="ExternalInput")
    bq_d = nc.dram_tensor("bq", [128, 1], F32, kind="ExternalInput")
    bk_d = nc.dram_tensor("bk", [128, 1], F32, kind="ExternalInput")
    bv_d = nc.dram_tensor("bv", [128], F32, kind="ExternalInput")
    id_d = nc.dram_tensor("idm", [128, 128], BF16, kind="ExternalInput")
    out_d = nc.dram_tensor("out", [S, D], F32, kind="ExternalOutput")

    with tile.TileContext(nc) as tc:
        with (
            tc.tile_pool(name="const", bufs=1) as cpool,
            tc.tile_pool(name="ptp", bufs=12) as ptpool,
            tc.tile_pool(name="wk2", bufs=2) as wpool,
            tc.tile_pool(name="ps", bufs=3, space="PSUM") as pspool,
        ):
            wq_s = cpool.tile([128, 4, 128], BF16, tag="wq")
            wk_s = cpool.tile([128, 4, 128], BF16, tag="wk")
            wv_s = cpool.tile([128, 4, 128], BF16, tag="wv")
            wo_s = cpool.tile([128, 512], BF16, tag="wo")
            bq_s = cpool.tile([128, 1], F32, tag="bq")
            bk_s = cpool.tile([128, 1], F32, tag="bk")
            bv_r = cpool.tile([128, 128], F32, tag="bvr")
            id_s = cpool.tile([128, 128], BF16, tag="idm")
            dum = cpool.tile([1, 4], F32, tag="dum")
            nc.vector.memset(dum[:], 0.0)
            nc.scalar.activation(dum[:], dum[:], Exp)  # hoist act-table load
            nc.sync.dma_start(wk_s[:], wk_d[:])
            nc.sync.dma_start(bk_s[:], bk_d[:])

            xA = cpool.tile([128, 4, S], BF16, tag="xA")
            kT2 = cpool.tile([128, S], F8E4, tag="kT2")
            qT2 = cpool.tile([128, S], F8E4, tag="qT2")
            kT8 = cpool.tile([32, 2, 2, S], F8E4, tag="kT8")
            qT8 = cpool.tile([32, 2, 2, S], F8E4, tag="qT8")
            vA = cpool.tile([128, NTP, 2, 2, E + 1], F8E4, tag="vA")
            yT8 = cpool.tile([128, NQC, 512], BF16, tag="yT8")
            nc.vector.memset(vA[:, :, :, :, E], 1.0)
            zro = cpool.tile([1, 260], BF16, tag="zro")
            nc.vector.memset(zro[:], 0.0)

            # ---- projections (K up front; Q/V interleaved into the sweep) --
            xt_v = xt_d[:].rearrange("(a b) t -> b a t", a=4)  # [128,4,S] view

            def kq_proj(ch, w_s, b_s, dst, pair=False):
                t0 = ch * TCH
                pk = pspool.tile([128, 2, 512], F32, tag="sc",
                                 name=f"pkq{ch}")
                n = 2 if pair else 1
                for i in range(n):
                    for ds_ in range(4):
                        nc.tensor.matmul(
                            pk[:, i, :], w_s[:, ds_, :],
                            xA[:, ds_, t0 + i * TCH:t0 + (i + 1) * TCH],
                            start=(ds_ == 0), stop=(ds_ == 3))
                nc.scalar.activation(dst[:, t0:t0 + n * TCH],
                                     pk[:, 0:n, :].rearrange("p a b -> p (a b)"),
                                     Ident, bias=b_s[:], scale=1.0)

            def rearr(src, dst, t0, tlen, eng=None):
                eng = eng or nc.sync
                for h in range(2):
                    for eh in range(2):
                        r0 = h * 64 + eh * 32
                        eng.dma_start(dst[0:32, eh, h, t0:t0 + tlen],
                                      src[r0:r0 + 32, t0:t0 + tlen])

            def v_piece(p):
                for ts_ in (2 * p, 2 * p + 1):
                    pv = pspool.tile([128, 512], F32, tag="ytp", bufs=1,
                                     name=f"pv{ts_}")
                    for ds_ in range(4):
                        nc.tensor.matmul(
                            pv[:, 0:128],
                            xA[:, ds_, ts_ * 128:(ts_ + 1) * 128],
                            wv_s[:, ds_, :], start=(ds_ == 0), stop=(ds_ == 3))
                    nc.vector.tensor_tensor(
                        out=vA[:, ts_ // 2, ts_ % 2, :, 0:E],
                        in0=pv[:, 0:128].rearrange("p (h e) -> p h e", h=2),
                        in1=bv_r[:].rearrange("p (h e) -> p h e", h=2),
                        op=ADD)

            def q_proj(ch):
                kq_proj(ch, wq_s, bq_s, qT2)
                rearr(qT2, qT8, ch * TCH, TCH)

            for ch in range(2):
                nc.sync.dma_start(xA[:, :, ch * TCH:(ch + 1) * TCH],
                                  xt_v[:, :, ch * TCH:(ch + 1) * TCH])
            nc.sync.dma_start(wq_s[:], wq_d[:])
            nc.sync.dma_start(bq_s[:], bq_d[:])
            nc.sync.dma_start(wv_s[:], wv_d[:])
            nc.sync.dma_start(
                bv_r[:], bv_d[:].unsqueeze(0).to_broadcast((128, 128)))
            nc.sync.dma_start(wo_s[:], wo_d[:])
            nc.sync.dma_start(id_s[:], id_d[:])
            for ch in range(2, 4):
                nc.sync.dma_start(xA[:, :, ch * TCH:(ch + 1) * TCH],
                                  xt_v[:, :, ch * TCH:(ch + 1) * TCH])
            for ch in range(0, 4, 2):
                kq_proj(ch, wk_s, bk_s, kT2, pair=True)
                if ch == 2:
                    rearr(kT2, kT8, 0, 2048)
            q_proj(0)
            for ch in range(4, NCH):
                nc.sync.dma_start(xA[:, :, ch * TCH:(ch + 1) * TCH],
                                  xt_v[:, :, ch * TCH:(ch + 1) * TCH])
            for ch in range(4, NCH, 2):
                kq_proj(ch, wk_s, bk_s, kT2, pair=True)
                if ch == 6:
                    rearr(kT2, kT8, 2048, 2048, eng=nc.gpsimd)

            # ---- phase 2: two head-phased sweeps, tails lagged one qc and
            # emitted in small pieces so no engine queue parks on the chain --
            def norm_piece(h, qc, pav_box):
                pav = pav_box["t"]
                rec = wpool.tile([128, 4], F32, tag="rec", name=f"rec{h}_{qc}")
                nc.vector.reciprocal(rec[:], pav[:, :, E])
                yN = wpool.tile([128, 4, E], BF16, tag="yN", name=f"yN{h}_{qc}")
                nc.vector.tensor_tensor(
                    out=yN[:], in0=pav[:, :, 0:E],
                    in1=rec[:, :, None].to_broadcast((128, 4, E)), op=MULT)
                return yN

            def transpose_piece(h, qc, yN, qbs):
                ytp = pspool.tile([128, 2, 128], BF16, tag="ytp",
                                  bufs=1, name=f"ytp{h}_{qc}_{qbs[0]}")
                for i, qb in enumerate(qbs):
                    nc.tensor.transpose(ytp[h * 64:(h + 1) * 64, i, :],
                                        yN[:, qb, :], id_s[:],
                                        tile_position=(0, h * 64))
                nc.vector.tensor_copy(
                    yT8[h * 64:(h + 1) * 64, qc,
                        qbs[0] * 128:(qbs[0] + 2) * 128],
                    ytp[h * 64:(h + 1) * 64, :, :].rearrange("p a b -> p (a b)"))

            out_v = out_d[:].rearrange("(qb p) d -> p qb d", p=128)

            def out_piece(qc, qbs, o_s):
                po = pspool.tile([128, 2, 512], F32, tag="sc",
                                 name=f"po{qc}_{qbs[0]}")
                for i, qb in enumerate(qbs):
                    nc.tensor.matmul(po[:, i, :],
                                     yT8[:, qc, qb * 128:(qb + 1) * 128],
                                     wo_s[:], start=True, stop=True)
                nc.scalar.copy(
                    o_s[:, qbs[0]:qbs[0] + 2, :].rearrange("p a b -> p (a b)"),
                    po[:].rearrange("p a b -> p (a b)"))
                if qbs[-1] == 3:
                    nc.sync.dma_start(
                        out_v[:, qc * 4:(qc + 1) * 4, :], o_s[:])

            def tail_pieces(h, qc, pav_box):
                # generator-style: each closure runs later, in pop order
                state = {}

                def p_norm():
                    state["yN"] = norm_piece(h, qc, pav_box)

                yield p_norm
                yield lambda: transpose_piece(h, qc, state["yN"], (0, 1))
                yield lambda: transpose_piece(h, qc, state["yN"], (2, 3))
                if h == 1:
                    def p_out1():
                        state["o_s"] = wpool.tile([128, 4, 512], F32,
                                                  tag="osb", name=f"os{qc}")
                        out_piece(qc, (0, 1), state["o_s"])

                    yield p_out1
                    yield lambda: out_piece(qc, (2, 3), state["o_s"])

            pending = []  # deferred tail pieces, popped one per even tp
            avq = []      # (pav, h, pt, tp) awaiting AV emission (lag 4)

            def emit_av(pav_box, h, ptp_, tpp):
                if "t" not in pav_box:
                    pav_box["t"] = pspool.tile(
                        [128, 4, E + 1], F32, tag="pav", bufs=1,
                        name=f"pav{h}_{pav_box['qc']}")
                    # explicit zero-fill: one whole-tile start=True matmul so
                    # the per-qb accumulation groups never re-zero each other
                    nc.tensor.matmul(
                        pav_box["t"][:].rearrange("p a b -> p (a b)"),
                        zro[0:1, 0:128], zro[0:1, 0:260],
                        start=True, stop=True)
                pav = pav_box["t"]
                for qb in range(4):
                    nc.tensor.matmul(
                        pav[:, qb, :], ptp_[:, :, qb * 128:(qb + 1) * 128],
                        vA[:, tpp, :, h, :],
                        start=False, stop=(tpp == NTP - 1), perf_mode=DR)

            for h in range(2):
                sched = _sweep_sched(ACT_N[h])
                ei = 0
                for qc in range(NQC):
                    q0 = qc * 512
                    pav_box = {"qc": qc}
                    for tp in range(NTP):
                        psc = pspool.tile([128, 2, 512], F32, tag="sc",
                                          name=f"psc{h}_{qc}_{tp}")
                        for i in range(2):
                            tt = tp * 2 + i
                            nc.tensor.matmul(
                                psc[:, i, :],
                                kT8[0:32, :, h, tt * 128:(tt + 1) * 128],
                                qT8[0:32, :, h, q0:q0 + 512],
                                start=True, stop=True, perf_mode=DR)
                        pt = ptpool.tile([128, 2, 512], F8E4, tag="pt",
                                         name=f"pt{h}_{qc}_{tp}")
                        if sched[ei] == "A":
                            nc.scalar.activation(pt[:], psc[:], Exp, scale=0.125)
                        else:
                            nc.vector.tensor_scalar(
                                pt[:].bitcast(I8), psc[:], SCH_MULT, SCH_BIAS,
                                MULT, ADD)
                        ei += 1
                        gtp = qc * NTP + tp
                        if h == 0 and gtp in V_AT:
                            for p in V_AT[gtp]:
                                v_piece(p)        # V feeds AV pops (lag 8)
                        if h == 0 and qc < NQC - 1 and tp == 9:
                            q_proj(qc + 1)        # Q chunk for the next qc
                        last = (h == 1 and qc == NQC - 1)
                        popat = (7, 8, 9, 10, 11) if last else (7, 9, 11, 13, 15)
                        if tp in popat and pending:
                            pending.pop(0)()
                        if len(avq) >= (2 if (last and tp > 8) else 7):
                            emit_av(*avq.pop(0))
                        if last and tp > 11 and avq:
                            emit_av(*avq.pop(0))
                        avq.append((pav_box, h, pt, tp))
                    pending.extend(tail_pieces(h, qc, pav_box))
            while avq:
                emit_av(*avq.pop(0))
            for fn in pending:
                fn()
    nc.compile()
    return nc


_NC = None


def kernel(x, Wq, bq, Wk, bk, Wv, bv, Wo, bo, **kw):
    global _NC
    x = np.asarray(x, np.float32)
    if _NC is None:
        _NC = build_program()

    idm = np.eye(128, dtype=NBF)
    xts = [np.ascontiguousarray(np.asarray(x[b], np.float32).T.astype(NBF))
           for b in range(B)]

    def wpack(W, hp):
        w = np.concatenate([np.asarray(W[2 * hp], np.float32),
                            np.asarray(W[2 * hp + 1], np.float32)], axis=1)
        return np.ascontiguousarray(
            w.reshape(4, 128, 128).transpose(1, 0, 2).astype(NBF))

    def bpack(bvec, hp):
        return np.ascontiguousarray(np.concatenate(
            [np.asarray(bvec[2 * hp], np.float32),
             np.asarray(bvec[2 * hp + 1], np.float32)]))

    in_maps = []
    for c in range(NCORES):
        b, hp = c // 4, c % 4
        in_maps.append({
            "xt": xts[b],
            "wq": wpack(Wq, hp), "wk": wpack(Wk, hp), "wv": wpack(Wv, hp),
            "wo": np.ascontiguousarray(
                np.asarray(Wo, np.float32)[2 * hp * 64:(2 * hp + 2) * 64]
                .astype(NBF)),
            "bq": bpack(bq, hp)[:, None], "bk": bpack(bk, hp)[:, None],
            "bv": bpack(bv, hp),
            "idm": idm,
        })
    res = run_bass_kernel_spmd(_NC, in_maps, core_ids=list(range(NCORES)))
    out = np.zeros((B, S, D), np.float32)
    for c in range(NCORES):
        out[c // 4] += res.results[c]["out"]
    out += np.asarray(bo, np.float32)[None, None, :]
    return out


# revision 39
# speedup vs baseline: 1.7774x; 1.0103x over previous
"""Multi-head attention Trainium2 kernel, 8-core SPMD, head+batch sharded.

Problem: x[2,4096,512], 8 heads of 64; per-head QKV proj, softmax(QK^T/8)V,
concat, output proj.

Sharding: core c = b*4 + hp handles batch b and head-pair (2hp, 2hp+1) over
the full 4096x4096 attention. Each core emits a PARTIAL output projection
(rows of Wo for its two heads); the host gather sums the 4 partials per batch
and adds bo (the all-reduce of the row-sharded projection done at unshard).

Structure (v3, sequencer/PSUM-aware):
  phase 1: x resident in SBUF (32KB bf16); K/Q/V projected per 512-t chunk
    (bf16 matmuls, 4-step d-contraction). K/Q evacuated by ACT (fused
    per-partition bias + fp8e4m3 convert), V by DVE (free-dim bias via
    tensor_tensor). K/Q then DMA-rearranged (Pool SWDGE, per-2048 blocks)
    into the DoubleRow layout [e%32][eh][h][t].
  phase 2: two head-phased sweeps (h=0 then h=1) over q-chunks of 512,
    t-pairs of 256 inner: scores = one fp8 DoubleRow matmul per 128-t tile
    (contraction 2x32); exp split ACT (exact, ->fp8) / DVE (Schraudolph:
    int8(z*8/ln2 + 55.65) bitcast fp8e4m3) -- Pool cannot read PSUM; AV via
    fp8 DoubleRow (contraction 2x128), V's ones-column accumulating the
    softmax denominator at PSUM row 64. Tail per (h,qc): DVE reciprocal +
    broadcast-multiply, PE transpose (identity matmul, h1 at tile_position
    col 64), DVE evac to yT8[he,q]; in the h1 sweep also the partial
    out-projection (contraction 128) + ACT evac + SP DMA out.

PSUM (8 banks): tag "sc" 3x[128,2,512]f32 (6 banks; also rotates phase-1
K/Q proj PSUM and tail po tiles) + "pav" 1 bank (zero-filled via an explicit
start=True matmul -- per-qb group starts would re-zero each other's bank) +
"ytp" 1 bank (V-proj PSUM early, transpose staging at tails).
"""

import numpy as np
import ml_dtypes

import concourse.bass as bass
from concourse import bacc
import concourse.mybir as mybir
import concourse.tile as tile
from concourse.bass_utils import run_bass_kernel_spmd

F32 = mybir.dt.float32
BF16 = mybir.dt.bfloat16
F8E4 = mybir.dt.float8e4
I8 = mybir.dt.int8
NBF = ml_dtypes.bfloat16

B, S, D, H, E = 2, 4096, 512, 8, 64
NCORES = 8
TCH = 512
NCH = S // TCH            # 8 chunks
NTP = S // 256            # 16 t-pairs per sweep
NQC = S // 512            # 8 q-chunks
LN2 = float(np.log(2.0))
SCH_MULT = 0.125 * 8.0 / LN2
SCH_BIAS = (7.0 - 0.0435) * 8.0

DR = mybir.MatmulPerfMode.DoubleRow
Exp = mybir.ActivationFunctionType.Exp
Ident = mybir.ActivationFunctionType.Identity
ADD = mybir.AluOpType.add
MULT = mybir.AluOpType.mult

# exp instructions per sweep assigned ACT vs DVE (weighted by per-instr cost
# 1038ns vs 1192ns, minus fixed non-exp engine load in each sweep)
ACT_N = {0: 74, 1: 64}
# V-projection pieces (2 of the 32 t-subtiles each) paced ~1.5 t-pairs apart
V_AT = {}
for _p in range(16):
    V_AT.setdefault(min(int(1.25 * _p), _p + 5), []).append(_p)


def _sweep_sched(n_act, total=128):
    acc_a = acc_d = 0.0
    fa, fd = n_act / total, (total - n_act) / total
    out = []
    for _ in range(total):
        acc_a += fa
        acc_d += fd
        if acc_a >= acc_d:
            out.append("A")
            acc_a -= 1.0
        else:
            out.append("D")
            acc_d -= 1.0
    return out


def build_program():
    nc = bacc.Bacc()
    xt_d = nc.dram_tensor("xt", [D, S], BF16, kind="ExternalInput")
    wq_d = nc.dram_tensor("wq", [128, 4, 128], BF16, kind="ExternalInput")
    wk_d = nc.dram_tensor("wk", [128, 4, 128], BF16, kind="ExternalInput")
    wv_d = nc.dram_tensor("wv", [128, 4, 128], BF16, kind="ExternalInput")
    wo_d = nc.dram_tensor("wo", [128, 512], BF16, kind="ExternalInput")
    bq_d = nc.dram_tensor("bq", [128, 1], F32, kind="ExternalInput")
    bk_d = nc.dram_tensor("bk", [128, 1], F32, kind="ExternalInput")
    bv_d = nc.dram_tensor("bv", [128], F32, kind="ExternalInput")
    id_d = nc.dram_tensor("idm", [128, 128], BF16, kind="ExternalInput")
    out_d = nc.dram_tensor("out", [S, D], BF16, kind="ExternalOutput")

    with tile.TileContext(nc) as tc:
        with (
            tc.tile_pool(name="const", bufs=1) as cpool,
            tc.tile_pool(name="ptp", bufs=12) as ptpool,
            tc.tile_pool(name="wk2", bufs=2) as wpool,
            tc.tile_pool(name="ps", bufs=3, space="PSUM") as pspool,
        ):
            wq_s = cpool.tile([128, 4, 128], BF16, tag="wq")
            wk_s = cpool.tile([128, 4, 128], BF16, tag="wk")
            wv_s = cpool.tile([128, 4, 128], BF16, tag="wv")
            wo_s = cpool.tile([128, 512], BF16, tag="wo")
            bq_s = cpool.tile([128, 1], F32, tag="bq")
            bk_s = cpool.tile([128, 1], F32, tag="bk")
            bv_r = cpool.tile([128, 128], F32, tag="bvr")
            id_s = cpool.tile([128, 128], BF16, tag="idm")
            dum = cpool.tile([1, 4], F32, tag="dum")
            nc.vector.memset(dum[:], 0.0)
            nc.scalar.activation(dum[:], dum[:], Exp)  # hoist act-table load
            nc.sync.dma_start(wk_s[:], wk_d[:])
            nc.sync.dma_start(bk_s[:], bk_d[:])

            xA = cpool.tile([128, 4, S], BF16, tag="xA")
            kT2 = cpool.tile([128, S], F8E4, tag="kT2")
            qT2 = cpool.tile([128, S], F8E4, tag="qT2")
            kT8 = cpool.tile([32, 2, 2, S], F8E4, tag="kT8")
            qT8 = cpool.tile([32, 2, 2, S], F8E4, tag="qT8")
            vA = cpool.tile([128, NTP, 2, 2, E + 1], F8E4, tag="vA")
            yT8 = cpool.tile([128, NQC, 512], BF16, tag="yT8")
            nc.vector.memset(vA[:, :, :, :, E], 1.0)
            zro = cpool.tile([1, 260], BF16, tag="zro")
            nc.vector.memset(zro[:], 0.0)

            # ---- projections (K up front; Q/V interleaved into the sweep) --
            xt_v = xt_d[:].rearrange("(a b) t -> b a t", a=4)  # [128,4,S] view

            def kq_proj(ch, w_s, b_s, dst, pair=False):
                t0 = ch * TCH
                pk = pspool.tile([128, 2, 512], F32, tag="sc",
                                 name=f"pkq{ch}")
                n = 2 if pair else 1
                for i in range(n):
                    for ds_ in range(4):
                        nc.tensor.matmul(
                            pk[:, i, :], w_s[:, ds_, :],
                            xA[:, ds_, t0 + i * TCH:t0 + (i + 1) * TCH],
                            start=(ds_ == 0), stop=(ds_ == 3))
                nc.scalar.activation(dst[:, t0:t0 + n * TCH],
                                     pk[:, 0:n, :].rearrange("p a b -> p (a b)"),
                                     Ident, bias=b_s[:], scale=1.0)

            def rearr(src, dst, t0, tlen, eng=None):
                eng = eng or nc.sync
                for h in range(2):
                    for eh in range(2):
                        r0 = h * 64 + eh * 32
                        eng.dma_start(dst[0:32, eh, h, t0:t0 + tlen],
                                      src[r0:r0 + 32, t0:t0 + tlen])

            def v_piece(p):
                for ts_ in (2 * p, 2 * p + 1):
                    pv = pspool.tile([128, 512], F32, tag="ytp", bufs=1,
                                     name=f"pv{ts_}")
                    for ds_ in range(4):
                        nc.tensor.matmul(
                            pv[:, 0:128],
                            xA[:, ds_, ts_ * 128:(ts_ + 1) * 128],
                            wv_s[:, ds_, :], start=(ds_ == 0), stop=(ds_ == 3))
                    nc.vector.tensor_tensor(
                        out=vA[:, ts_ // 2, ts_ % 2, :, 0:E],
                        in0=pv[:, 0:128].rearrange("p (h e) -> p h e", h=2),
                        in1=bv_r[:].rearrange("p (h e) -> p h e", h=2),
                        op=ADD)

            def q_proj(ch):
                kq_proj(ch, wq_s, bq_s, qT2)
                rearr(qT2, qT8, ch * TCH, TCH)

            for ch in range(2):
                nc.sync.dma_start(xA[:, :, ch * TCH:(ch + 1) * TCH],
                                  xt_v[:, :, ch * TCH:(ch + 1) * TCH])
            nc.sync.dma_start(wq_s[:], wq_d[:])
            nc.sync.dma_start(bq_s[:], bq_d[:])
            nc.sync.dma_start(wv_s[:], wv_d[:])
            nc.sync.dma_start(
                bv_r[:], bv_d[:].unsqueeze(0).to_broadcast((128, 128)))
            nc.sync.dma_start(wo_s[:], wo_d[:])
            nc.sync.dma_start(id_s[:], id_d[:])
            for ch in range(2, 4):
                nc.sync.dma_start(xA[:, :, ch * TCH:(ch + 1) * TCH],
                                  xt_v[:, :, ch * TCH:(ch + 1) * TCH])
            for ch in range(0, 4, 2):
                kq_proj(ch, wk_s, bk_s, kT2, pair=True)
                if ch == 2:
                    rearr(kT2, kT8, 0, 2048)
            q_proj(0)
            for ch in range(4, NCH):
                nc.sync.dma_start(xA[:, :, ch * TCH:(ch + 1) * TCH],
                                  xt_v[:, :, ch * TCH:(ch + 1) * TCH])
            for ch in range(4, NCH, 2):
                kq_proj(ch, wk_s, bk_s, kT2, pair=True)
                if ch == 6:
                    rearr(kT2, kT8, 2048, 2048, eng=nc.gpsimd)

            # ---- phase 2: two head-phased sweeps, tails lagged one qc and
            # emitted in small pieces so no engine queue parks on the chain --
            def norm_piece(h, qc, pav_box):
                pav = pav_box["t"]
                rec = wpool.tile([128, 4], F32, tag="rec", name=f"rec{h}_{qc}")
                nc.vector.reciprocal(rec[:], pav[:, :, E])
                yN = wpool.tile([128, 4, E], BF16, tag="yN", name=f"yN{h}_{qc}")
                nc.vector.tensor_tensor(
                    out=yN[:], in0=pav[:, :, 0:E],
                    in1=rec[:, :, None].to_broadcast((128, 4, E)), op=MULT)
                return yN

            def transpose_piece(h, qc, yN, qbs):
                ytp = pspool.tile([128, 2, 128], BF16, tag="ytp",
                                  bufs=1, name=f"ytp{h}_{qc}_{qbs[0]}")
                for i, qb in enumerate(qbs):
                    nc.tensor.transpose(ytp[h * 64:(h + 1) * 64, i, :],
                                        yN[:, qb, :], id_s[:],
                                        tile_position=(0, h * 64))
                nc.vector.tensor_copy(
                    yT8[h * 64:(h + 1) * 64, qc,
                        qbs[0] * 128:(qbs[0] + 2) * 128],
                    ytp[h * 64:(h + 1) * 64, :, :].rearrange("p a b -> p (a b)"))

            out_v = out_d[:].rearrange("(qb p) d -> p qb d", p=128)

            def out_piece(qc, qbs, o_s):
                po = pspool.tile([128, 2, 512], F32, tag="sc",
                                 name=f"po{qc}_{qbs[0]}")
                for i, qb in enumerate(qbs):
                    nc.tensor.matmul(po[:, i, :],
                                     yT8[:, qc, qb * 128:(qb + 1) * 128],
                                     wo_s[:], start=True, stop=True)
                nc.scalar.copy(
                    o_s[:, qbs[0]:qbs[0] + 2, :].rearrange("p a b -> p (a b)"),
                    po[:].rearrange("p a b -> p (a b)"))
                if qc == NQC - 1:
                    nc.sync.dma_start(
                        out_v[:, qc * 4 + qbs[0]:qc * 4 + qbs[0] + 2, :],
                        o_s[:, qbs[0]:qbs[0] + 2, :])
                elif qbs[-1] == 3:
                    nc.sync.dma_start(
                        out_v[:, qc * 4:(qc + 1) * 4, :], o_s[:])

            def tail_pieces(h, qc, pav_box):
                # generator-style: each closure runs later, in pop order
                state = {}

                def p_norm():
                    state["yN"] = norm_piece(h, qc, pav_box)

                yield p_norm
                yield lambda: transpose_piece(h, qc, state["yN"], (0, 1))
                yield lambda: transpose_piece(h, qc, state["yN"], (2, 3))
                if h == 1:
                    def p_out1():
                        state["o_s"] = wpool.tile([128, 4, 512], BF16,
                                                  tag="osb", name=f"os{qc}")
                        out_piece(qc, (0, 1), state["o_s"])

                    yield p_out1
                    yield lambda: out_piece(qc, (2, 3), state["o_s"])

            pending = []  # deferred tail pieces, popped one per even tp
            avq = []      # (pav, h, pt, tp) awaiting AV emission (lag 4)

            def emit_av(pav_box, h, ptp_, tpp):
                if "t" not in pav_box:
                    pav_box["t"] = pspool.tile(
                        [128, 4, E + 1], F32, tag="pav", bufs=1,
                        name=f"pav{h}_{pav_box['qc']}")
                    # explicit zero-fill: one whole-tile start=True matmul so
                    # the per-qb accumulation groups never re-zero each other
                    nc.tensor.matmul(
                        pav_box["t"][:].rearrange("p a b -> p (a b)"),
                        zro[0:1, 0:128], zro[0:1, 0:260],
                        start=True, stop=True)
                pav = pav_box["t"]
                for qb in range(4):
                    nc.tensor.matmul(
                        pav[:, qb, :], ptp_[:, :, qb * 128:(qb + 1) * 128],
                        vA[:, tpp, :, h, :],
                        start=False, stop=(tpp == NTP - 1), perf_mode=DR)

            for h in range(2):
                sched = _sweep_sched(ACT_N[h])
                ei = 0
                for qc in range(NQC):
                    q0 = qc * 512
                    pav_box = {"qc": qc}
                    for tp in range(NTP):
                        psc = pspool.tile([128, 2, 512], F32, tag="sc",
                                          name=f"psc{h}_{qc}_{tp}")
                        for i in range(2):
                            tt = tp * 2 + i
                            nc.tensor.matmul(
                                psc[:, i, :],
                                kT8[0:32, :, h, tt * 128:(tt + 1) * 128],
                                qT8[0:32, :, h, q0:q0 + 512],
                                start=True, stop=True, perf_mode=DR)
                        pt = ptpool.tile([128, 2, 512], F8E4, tag="pt",
                                         name=f"pt{h}_{qc}_{tp}")
                        if sched[ei] == "A":
                            nc.scalar.activation(pt[:], psc[:], Exp, scale=0.125)
                        else:
                            nc.vector.tensor_scalar(
                                pt[:].bitcast(I8), psc[:], SCH_MULT, SCH_BIAS,
                                MULT, ADD)
                        ei += 1
                        gtp = qc * NTP + tp
                        if h == 0 and gtp in V_AT:
                            for p in V_AT[gtp]:
                                v_piece(p)        # V feeds AV pops (lag 8)
                        if h == 0 and qc < NQC - 1 and tp == 9:
                            q_proj(qc + 1)        # Q chunk for the next qc
                        last = (h == 1 and qc == NQC - 1)
                        popat = (7, 8, 9, 10, 11) if last else (7, 9, 11, 13, 15)
                        if tp in popat and pending:
                            pending.pop(0)()
                        if len(avq) >= (2 if (last and tp > 8) else 7):
                            emit_av(*avq.pop(0))
                        if last and tp > 11 and avq:
                            emit_av(*avq.pop(0))
                        avq.append((pav_box, h, pt, tp))
                    pending.extend(tail_pieces(h, qc, pav_box))
            while avq:
                emit_av(*avq.pop(0))
            for fn in pending:
                fn()
    nc.compile()
    return nc


_NC = None


def kernel(x, Wq, bq, Wk, bk, Wv, bv, Wo, bo, **kw):
    global _NC
    x = np.asarray(x, np.float32)
    if _NC is None:
        _NC = build_program()

    idm = np.eye(128, dtype=NBF)
    xts = [np.ascontiguousarray(np.asarray(x[b], np.float32).T.astype(NBF))
           for b in range(B)]

    def wpack(W, hp):
        w = np.concatenate([np.asarray(W[2 * hp], np.float32),
                            np.asarray(W[2 * hp + 1], np.float32)], axis=1)
        return np.ascontiguousarray(
            w.reshape(4, 128, 128).transpose(1, 0, 2).astype(NBF))

    def bpack(bvec, hp):
        return np.ascontiguousarray(np.concatenate(
            [np.asarray(bvec[2 * hp], np.float32),
             np.asarray(bvec[2 * hp + 1], np.float32)]))

    in_maps = []
    for c in range(NCORES):
        b, hp = c // 4, c % 4
        in_maps.append({
            "xt": xts[b],
            "wq": wpack(Wq, hp), "wk": wpack(Wk, hp), "wv": wpack(Wv, hp),
            "wo": np.ascontiguousarray(
                np.asarray(Wo, np.float32)[2 * hp * 64:(2 * hp + 2) * 64]
                .astype(NBF)),
            "bq": bpack(bq, hp)[:, None], "bk": bpack(bk, hp)[:, None],
            "bv": bpack(bv, hp),
            "idm": idm,
        })
    res = run_bass_kernel_spmd(_NC, in_maps, core_ids=list(range(NCORES)))
    out = np.zeros((B, S, D), np.float32)
    for c in range(NCORES):
        out[c // 4] += np.asarray(res.results[c]["out"]).astype(np.float32)
    out += np.asarray(bo, np.float32)[None, None, :]
    return out


# revision 41
# speedup vs baseline: 1.7828x; 1.0031x over previous
"""Multi-head attention Trainium2 kernel, 8-core SPMD, head+batch sharded.

Problem: x[2,4096,512], 8 heads of 64; per-head QKV proj, softmax(QK^T/8)V,
concat, output proj.

Sharding: core c = b*4 + hp handles batch b and head-pair (2hp, 2hp+1) over
the full 4096x4096 attention. Each core emits a PARTIAL output projection
(rows of Wo for its two heads); the host gather sums the 4 partials per batch
and adds bo (the all-reduce of the row-sharded projection done at unshard).

Structure (v3, sequencer/PSUM-aware):
  phase 1: x resident in SBUF (32KB bf16); K/Q/V projected per 512-t chunk
    (bf16 matmuls, 4-step d-contraction). K/Q evacuated by ACT (fused
    per-partition bias + fp8e4m3 convert), V by DVE (free-dim bias via
    tensor_tensor). K/Q then DMA-rearranged (Pool SWDGE, per-2048 blocks)
    into the DoubleRow layout [e%32][eh][h][t].
  phase 2: two head-phased sweeps (h=0 then h=1) over q-chunks of 512,
    t-pairs of 256 inner: scores = one fp8 DoubleRow matmul per 128-t tile
    (contraction 2x32); exp split ACT (exact, ->fp8) / DVE (Schraudolph:
    int8(z*8/ln2 + 55.65) bitcast fp8e4m3) -- Pool cannot read PSUM; AV via
    fp8 DoubleRow (contraction 2x128), V's ones-column accumulating the
    softmax denominator at PSUM row 64. Tail per (h,qc): DVE reciprocal +
    broadcast-multiply, PE transpose (identity matmul, h1 at tile_position
    col 64), DVE evac to yT8[he,q]; in the h1 sweep also the partial
    out-projection (contraction 128) + ACT evac + SP DMA out.

PSUM (8 banks): tag "sc" 3x[128,2,512]f32 (6 banks; also rotates phase-1
K/Q proj PSUM and tail po tiles) + "pav" 1 bank (zero-filled via an explicit
start=True matmul -- per-qb group starts would re-zero each other's bank) +
"ytp" 1 bank (V-proj PSUM early, transpose staging at tails).
"""

import numpy as np
import ml_dtypes

import concourse.bass as bass
from concourse import bacc
import concourse.mybir as mybir
import concourse.tile as tile
from concourse.bass_utils import run_bass_kernel_spmd

F32 = mybir.dt.float32
BF16 = mybir.dt.bfloat16
F8E4 = mybir.dt.float8e4
I8 = mybir.dt.int8
NBF = ml_dtypes.bfloat16

B, S, D, H, E = 2, 4096, 512, 8, 64
NCORES = 8
TCH = 512
NCH = S // TCH            # 8 chunks
NTP = S // 256            # 16 t-pairs per sweep
NQC = S // 512            # 8 q-chunks
LN2 = float(np.log(2.0))
SCH_MULT = 0.125 * 8.0 / LN2
SCH_BIAS = (7.0 - 0.0435) * 8.0

DR = mybir.MatmulPerfMode.DoubleRow
Exp = mybir.ActivationFunctionType.Exp
Ident = mybir.ActivationFunctionType.Identity
ADD = mybir.AluOpType.add
MULT = mybir.AluOpType.mult

# exp instructions per sweep assigned ACT vs DVE (weighted by per-instr cost
# 1038ns vs 1192ns, minus fixed non-exp engine load in each sweep)
ACT_N = {0: 74, 1: 64}
# V-projection pieces (2 of the 32 t-subtiles each) paced ~1.5 t-pairs apart
V_AT = {}
for _p in range(16):
    V_AT.setdefault(min(int(1.25 * _p), _p + 5), []).append(_p)


def _sweep_sched(n_act, total=128):
    acc_a = acc_d = 0.0
    fa, fd = n_act / total, (total - n_act) / total
    out = []
    for _ in range(total):
        acc_a += fa
        acc_d += fd
        if acc_a >= acc_d:
            out.append("A")
            acc_a -= 1.0
        else:
            out.append("D")
            acc_d -= 1.0
    return out


def build_program():
    nc = bacc.Bacc()
    xt_d = nc.dram_tensor("xt", [D, S], BF16, kind="ExternalInput")
    wq_d = nc.dram_tensor("wq", [128, 4, 128], BF16, kind="ExternalInput")
    wk_d = nc.dram_tensor("wk", [128, 4, 128], BF16, kind="ExternalInput")
    wv_d = nc.dram_tensor("wv", [128, 4, 128], BF16, kind="ExternalInput")
    wo_d = nc.dram_tensor("wo", [128, 512], BF16, kind="ExternalInput")
    bq_d = nc.dram_tensor("bq", [128, 1], F32, kind="ExternalInput")
    bk_d = nc.dram_tensor("bk", [128, 1], F32, kind="ExternalInput")
    bv_d = nc.dram_tensor("bv", [128], F32, kind="ExternalInput")
    id_d = nc.dram_tensor("idm", [128, 128], BF16, kind="ExternalInput")
    out_d = nc.dram_tensor("out", [S, D], BF16, kind="ExternalOutput")

    with tile.TileContext(nc) as tc:
        with (
            tc.tile_pool(name="const", bufs=1) as cpool,
            tc.tile_pool(name="ptp", bufs=12) as ptpool,
            tc.tile_pool(name="wk2", bufs=2) as wpool,
            tc.tile_pool(name="ps", bufs=3, space="PSUM") as pspool,
        ):
            wq_s = cpool.tile([128, 4, 128], BF16, tag="wq")
            wk_s = cpool.tile([128, 4, 128], BF16, tag="wk")
            wv_s = cpool.tile([128, 4, 128], BF16, tag="wv")
            wo_s = cpool.tile([128, 512], BF16, tag="wo")
            bq_s = cpool.tile([128, 1], F32, tag="bq")
            bk_s = cpool.tile([128, 1], F32, tag="bk")
            bv_r = cpool.tile([128, 128], F32, tag="bvr")
            id_s = cpool.tile([128, 128], BF16, tag="idm")
            dum = cpool.tile([1, 4], F32, tag="dum")
            nc.vector.memset(dum[:], 0.0)
            nc.scalar.activation(dum[:], dum[:], Exp)  # hoist act-table load
            nc.sync.dma_start(wk_s[:], wk_d[:])
            nc.sync.dma_start(bk_s[:], bk_d[:])

            xA = cpool.tile([128, 4, S], BF16, tag="xA")
            kT2 = cpool.tile([128, S], F8E4, tag="kT2")
            qT2 = cpool.tile([128, S], F8E4, tag="qT2")
            kT8 = cpool.tile([32, 2, 2, S], F8E4, tag="kT8")
            qT8 = cpool.tile([32, 2, 2, S], F8E4, tag="qT8")
            vA = cpool.tile([128, NTP, 2, 2, E + 1], F8E4, tag="vA")
            yT8 = cpool.tile([128, NQC, 512], BF16, tag="yT8")
            nc.vector.memset(vA[:, :, :, :, E], 1.0)
            zro = cpool.tile([1, 260], BF16, tag="zro")
            nc.vector.memset(zro[:], 0.0)

            # ---- projections (K up front; Q/V interleaved into the sweep) --
            xt_v = xt_d[:].rearrange("(a b) t -> b a t", a=4)  # [128,4,S] view

            def kq_proj(ch, w_s, b_s, dst, pair=False):
                t0 = ch * TCH
                pk = pspool.tile([128, 2, 512], F32, tag="sc",
                                 name=f"pkq{ch}")
                n = 2 if pair else 1
                for i in range(n):
                    for ds_ in range(4):
                        nc.tensor.matmul(
                            pk[:, i, :], w_s[:, ds_, :],
                            xA[:, ds_, t0 + i * TCH:t0 + (i + 1) * TCH],
                            start=(ds_ == 0), stop=(ds_ == 3))
                nc.scalar.activation(dst[:, t0:t0 + n * TCH],
                                     pk[:, 0:n, :].rearrange("p a b -> p (a b)"),
                                     Ident, bias=b_s[:], scale=1.0)

            def rearr(src, dst, t0, tlen, eng=None):
                eng = eng or nc.sync
                for h in range(2):
                    for eh in range(2):
                        r0 = h * 64 + eh * 32
                        eng.dma_start(dst[0:32, eh, h, t0:t0 + tlen],
                                      src[r0:r0 + 32, t0:t0 + tlen])

            def v_piece(p):
                for ts_ in (2 * p, 2 * p + 1):
                    pv = pspool.tile([128, 512], F32, tag="ytp", bufs=1,
                                     name=f"pv{ts_}")
                    for ds_ in range(4):
                        nc.tensor.matmul(
                            pv[:, 0:128],
                            xA[:, ds_, ts_ * 128:(ts_ + 1) * 128],
                            wv_s[:, ds_, :], start=(ds_ == 0), stop=(ds_ == 3))
                    nc.vector.tensor_tensor(
                        out=vA[:, ts_ // 2, ts_ % 2, :, 0:E],
                        in0=pv[:, 0:128].rearrange("p (h e) -> p h e", h=2),
                        in1=bv_r[:].rearrange("p (h e) -> p h e", h=2),
                        op=ADD)

            def q_proj(ch):
                kq_proj(ch, wq_s, bq_s, qT2)
                rearr(qT2, qT8, ch * TCH, TCH)

            for ch in range(2):
                nc.sync.dma_start(xA[:, :, ch * TCH:(ch + 1) * TCH],
                                  xt_v[:, :, ch * TCH:(ch + 1) * TCH])
            nc.sync.dma_start(wq_s[:], wq_d[:])
            nc.sync.dma_start(bq_s[:], bq_d[:])
            nc.sync.dma_start(wv_s[:], wv_d[:])
            nc.sync.dma_start(
                bv_r[:], bv_d[:].unsqueeze(0).to_broadcast((128, 128)))
            nc.sync.dma_start(wo_s[:], wo_d[:])
            nc.sync.dma_start(id_s[:], id_d[:])
            for ch in range(2, 4):
                nc.sync.dma_start(xA[:, :, ch * TCH:(ch + 1) * TCH],
                                  xt_v[:, :, ch * TCH:(ch + 1) * TCH])
            for ch in range(0, 4, 2):
                kq_proj(ch, wk_s, bk_s, kT2, pair=True)
                if ch == 2:
                    rearr(kT2, kT8, 0, 2048)
            q_proj(0)
            for ch in range(4, NCH):
                nc.sync.dma_start(xA[:, :, ch * TCH:(ch + 1) * TCH],
                                  xt_v[:, :, ch * TCH:(ch + 1) * TCH])
            for ch in range(4, NCH, 2):
                kq_proj(ch, wk_s, bk_s, kT2, pair=True)
                if ch == 6:
                    rearr(kT2, kT8, 2048, 2048, eng=nc.gpsimd)

            # ---- phase 2: two head-phased sweeps, tails lagged one qc and
            # emitted in small pieces so no engine queue parks on the chain --
            def norm_piece(h, qc, pav_box):
                pav = pav_box["t"]
                rec = wpool.tile([128, 4], F32, tag="rec", name=f"rec{h}_{qc}")
                nc.vector.reciprocal(rec[:], pav[:, :, E])
                yN = wpool.tile([128, 4, E], BF16, tag="yN", name=f"yN{h}_{qc}")
                nc.vector.tensor_tensor(
                    out=yN[:], in0=pav[:, :, 0:E],
                    in1=rec[:, :, None].to_broadcast((128, 4, E)), op=MULT)
                return yN

            def transpose_piece(h, qc, yN, qbs):
                ytp = pspool.tile([128, 2, 128], BF16, tag="ytp",
                                  bufs=1, name=f"ytp{h}_{qc}_{qbs[0]}")
                for i, qb in enumerate(qbs):
                    nc.tensor.transpose(ytp[h * 64:(h + 1) * 64, i, :],
                                        yN[:, qb, :], id_s[:],
                                        tile_position=(0, h * 64))
                nc.vector.tensor_copy(
                    yT8[h * 64:(h + 1) * 64, qc,
                        qbs[0] * 128:(qbs[0] + 2) * 128],
                    ytp[h * 64:(h + 1) * 64, :, :].rearrange("p a b -> p (a b)"))

            out_v = out_d[:].rearrange("(qb p) d -> p qb d", p=128)

            def out_piece(qc, qbs, o_s):
                po = pspool.tile([128, 2, 512], F32, tag="sc",
                                 name=f"po{qc}_{qbs[0]}")
                for i, qb in enumerate(qbs):
                    nc.tensor.matmul(po[:, i, :],
                                     yT8[:, qc, qb * 128:(qb + 1) * 128],
                                     wo_s[:], start=True, stop=True)
                nc.scalar.copy(
                    o_s[:, qbs[0]:qbs[0] + 2, :].rearrange("p a b -> p (a b)"),
                    po[:].rearrange("p a b -> p (a b)"))
                if qc == NQC - 1:
                    nc.sync.dma_start(
                        out_v[:, qc * 4 + qbs[0]:qc * 4 + qbs[0] + 2, :],
                        o_s[:, qbs[0]:qbs[0] + 2, :])
                elif qbs[-1] == 3:
                    nc.sync.dma_start(
                        out_v[:, qc * 4:(qc + 1) * 4, :], o_s[:])

            def tail_pieces(h, qc, pav_box):
                # generator-style: each closure runs later, in pop order
                state = {}

                def p_norm():
                    state["yN"] = norm_piece(h, qc, pav_box)

                yield p_norm
                yield lambda: transpose_piece(h, qc, state["yN"], (0, 1))
                yield lambda: transpose_piece(h, qc, state["yN"], (2, 3))
                if h == 1:
                    def p_out1():
                        state["o_s"] = wpool.tile([128, 4, 512], BF16,
                                                  tag="osb", name=f"os{qc}")
                        out_piece(qc, (0, 1), state["o_s"])

                    yield p_out1
                    yield lambda: out_piece(qc, (2, 3), state["o_s"])

            pending = []  # deferred tail pieces, popped one per even tp
            avq = []      # (pav, h, pt, tp) awaiting AV emission (lag 4)

            def emit_av(pav_box, h, ptp_, tpp):
                if "t" not in pav_box:
                    pav_box["t"] = pspool.tile(
                        [128, 4, E + 1], F32, tag="pav", bufs=1,
                        name=f"pav{h}_{pav_box['qc']}")
                    # explicit zero-fill: one whole-tile start=True matmul so
                    # the per-qb accumulation groups never re-zero each other
                    nc.tensor.matmul(
                        pav_box["t"][:].rearrange("p a b -> p (a b)"),
                        zro[0:1, 0:128], zro[0:1, 0:260],
                        start=True, stop=True)
                pav = pav_box["t"]
                for qb in range(4):
                    nc.tensor.matmul(
                        pav[:, qb, :], ptp_[:, :, qb * 128:(qb + 1) * 128],
                        vA[:, tpp, :, h, :],
                        start=False, stop=(tpp == NTP - 1), perf_mode=DR)

            for h in range(2):
                sched = _sweep_sched(ACT_N[h])
                ei = 0
                for qc in range(NQC):
                    q0 = qc * 512
                    pav_box = {"qc": qc}
                    for tp in range(NTP):
                        psc = pspool.tile([128, 2, 512], F32, tag="sc",
                                          name=f"psc{h}_{qc}_{tp}")
                        for i in range(2):
                            tt = tp * 2 + i
                            nc.tensor.matmul(
                                psc[:, i, :],
                                kT8[0:32, :, h, tt * 128:(tt + 1) * 128],
                                qT8[0:32, :, h, q0:q0 + 512],
                                start=True, stop=True, perf_mode=DR)
                        pt = ptpool.tile([128, 2, 512], F8E4, tag="pt",
                                         name=f"pt{h}_{qc}_{tp}")
                        if sched[ei] == "A":
                            nc.scalar.activation(pt[:], psc[:], Exp, scale=0.125)
                        else:
                            nc.vector.tensor_scalar(
                                pt[:].bitcast(I8), psc[:], SCH_MULT, SCH_BIAS,
                                MULT, ADD)
                        ei += 1
                        gtp = qc * NTP + tp
                        if h == 0 and gtp in V_AT:
                            for p in V_AT[gtp]:
                                v_piece(p)        # V feeds AV pops (lag 8)
                        if h == 0 and qc < NQC - 1 and tp == 9:
                            q_proj(qc + 1)        # Q chunk for the next qc
                        last = (h == 1 and qc == NQC - 1)
                        popat = (8, 9, 10, 11, 12) if last else (8, 10, 12, 14, 15)
                        if tp in popat and pending:
                            pending.pop(0)()
                        if len(avq) >= (2 if (last and tp > 8) else 8):
                            emit_av(*avq.pop(0))
                        if last and tp > 11 and avq:
                            emit_av(*avq.pop(0))
                        avq.append((pav_box, h, pt, tp))
                    pending.extend(tail_pieces(h, qc, pav_box))
            while avq:
                emit_av(*avq.pop(0))
            for fn in pending:
                fn()
    nc.compile()
    return nc


_NC = None


def kernel(x, Wq, bq, Wk, bk, Wv, bv, Wo, bo, **kw):
    global _NC
    x = np.asarray(x, np.float32)
    if _NC is None:
        _NC = build_program()

    idm = np.eye(128, dtype=NBF)
    xts = [np.ascontiguousarray(np.asarray(x[b], np.float32).T.astype(NBF))
           for b in range(B)]

    def wpack(W, hp):
        w = np.concatenate([np.asarray(W[2 * hp], np.float32),
                            np.asarray(W[2 * hp + 1], np.float32)], axis=1)
        return np.ascontiguousarray(
            w.reshape(4, 128, 128).transpose(1, 0, 2).astype(NBF))

    def bpack(bvec, hp):
        return np.ascontiguousarray(np.concatenate(
            [np.asarray(bvec[2 * hp], np.float32),
             np.asarray(bvec[2 * hp + 1], np.float32)]))

    in_maps = []
    for c in range(NCORES):
        b, hp = c // 4, c % 4
        in_maps.append({
            "xt": xts[b],
            "wq": wpack(Wq, hp), "wk": wpack(Wk, hp), "wv": wpack(Wv, hp),
            "wo": np.ascontiguousarray(
                np.asarray(Wo, np.float32)[2 * hp * 64:(2 * hp + 2) * 64]
                .astype(NBF)),
            "bq": bpack(bq, hp)[:, None], "bk": bpack(bk, hp)[:, None],
            "bv": bpack(bv, hp),
            "idm": idm,
        })
    res = run_bass_kernel_spmd(_NC, in_maps, core_ids=list(range(NCORES)))
    out = np.zeros((B, S, D), np.float32)
    for c in range(NCORES):
        out[c // 4] += np.asarray(res.results[c]["out"]).astype(np.float32)
    out += np.asarray(bo, np.float32)[None, None, :]
    return out
